# revision 6
# baseline (speedup 1.0000x reference)
"""Trainium2 Bass kernel for nn_MixedDecoder (moe_routing).

Math (matches the reference exactly): only the LAST expert layer matters —
the reference never feeds layer outputs back into `z`, so layers 0/1 are
dead code.  Computed per sample b:
    coef = softmax(gate_mlp(z))                        # [B, 8]
    out  = sum_e coef[b,e] * (z @ w2[e]) + coef @ b2   # [B, 256]

Sharding: data-parallel over batch B=2048 across 8 cores (256 rows/core),
weights replicated.  All matmul operands are bf16.  The K=288 contraction
is packed as 128+128+32: the two full chunks run as [K=128,N=512] expert-
pair matmuls, and the four K=32 leftovers run CONCURRENTLY as row-tiled
matmuls at tile_position (32p, 0) — one pass instead of four.  ELU is
relu(x)+min(exp(x),1) with the "+1" folded into host-adjusted biases; the
two pieces are combined on the (otherwise idle) GPSIMD engine so each gate
layer is a single matmul.  Per-expert coefficient scaling happens on PSUM
eviction (per-partition scalar, split ACT/DVE), then a PE re-sum of
identity-matmuls + the mixed-bias matmul accumulates the experts, and the
final eviction applies the softmax 1/sum.  Inputs arrive in 5 merged DMAs
(tiny f32 biases; gate+zT pack; 3 w2 pieces) to cut HWDGE serialization.
"""

import numpy as np
import ml_dtypes

N_CORES = 8
B = 2048
IN_SIZE = 288
HIDDEN = 256
E = 8
GATE_H = 64
OUT_SIZE = 256
BL = B // N_CORES          # 256 rows per core
NCH = BL // 128            # 2 batch chunks of 128

# inp pack column offsets (bf16 [128, INPX])
ZT0 = 0                    # zT rows 0:128      [128, 256]
ZT1 = ZT0 + BL             # zT rows 128:256    [128, 256]
ZT2 = ZT1 + BL             # zT rows 256:288 replicated x4: [32p+r] = row 256+r
G0W = ZT2 + BL             # g0_w chunks: [128,64] [128,64] [32,64 (pad)]
G1W = G0W + 3 * GATE_H     # g1_w [64, 64]
G2W = G1W + GATE_H         # g2_w [64, 8]
SMC = G2W + E              # f32 bias pack, bit-cast as 24 bf16 cols:
                           # col0 g0_b | col1 b1_adj | col2[0:8] adj2 |
                           # row0 cols 3:11 adj2
INPX = SMC + 24            # 1056

# w2 pack column offsets (bf16 [128, W2X]); pair p = experts (2p, 2p+1)
# chunk0: pair-major [128, 4 x 512]; chunk1: same; chunk2: [32p+r] = pair p
# row 256+r [128, 512]; b2 at rows 0:8 [8, 256] (rows 8:128 padding)
W2C0 = 0
W2C1 = W2C0 + 4 * 2 * OUT_SIZE     # 2048
W2C2 = W2C1 + 4 * 2 * OUT_SIZE     # 4096
B2C = W2C2 + 2 * OUT_SIZE          # 4608
W2X = B2C + OUT_SIZE               # 4864

_CACHE = {}


def _build_nc(reps=1):
    from concourse import bacc
    import concourse.mybir as mybir
    from concourse.tile import TileContext
    from concourse.masks import make_identity

    dt = mybir.dt
    F32 = dt.float32
    BF = dt.bfloat16
    AF = mybir.ActivationFunctionType
    OP = mybir.AluOpType

    nc = bacc.Bacc("TRN2", target_bir_lowering=False, debug=False)

    inp_d = nc.declare_dram_parameter("inp", [128, INPX], BF, isOutput=False)
    w2_d = nc.declare_dram_parameter("w2p", [128, W2X], BF, isOutput=False)
    out_d = nc.declare_dram_parameter("outp", [128, NCH * OUT_SIZE], BF,
                                      isOutput=True)

    with TileContext(nc) as tc:
        with (
            tc.tile_pool(name="const", bufs=1) as cp,
            tc.tile_pool(name="w2p", bufs=3) as wp,
            tc.tile_pool(name="inp", bufs=3) as ip,
            tc.tile_pool(name="wk", bufs=3) as wk,
            tc.tile_pool(name="py", bufs=6, space="PSUM") as py,
            tc.tile_pool(name="pg", bufs=2, space="PSUM") as pg,
        ):
            # ---------------- hoisted constants (once per NEFF) -------------
            ident = cp.tile([128, 128], BF, name="ident")
            make_identity(nc, ident[:])
            ones_row = cp.tile([1, 128], F32, name="ones_row")
            nc.vector.memset(ones_row[:], 1.0)

            # dummy exp so the ACT Exp-table load happens before it's needed
            warm = cp.tile([1, 1], F32, name="warm")
            nc.vector.memset(warm[:], 0.0)
            warm2 = cp.tile([1, 1], F32, name="warm2")
            nc.scalar.activation(warm2[:], warm[:], AF.Exp)

            # PE warm-up: dummy matmuls so the HAM clock-gate releases before
            # the real work arrives (throwaway results)
            wu_ps = py.tile([128, 128], F32, name="wups", tag="py")
            for _ in range(8):
                nc.tensor.matmul(wu_ps[:], ident[:], ident[:],
                                 start=True, stop=True)

            for _rep in range(reps):
                # ------- DMAs: gate-critical first, then w2 pieces ----------
                inp_r = ip.tile([128, INPX], BF, name="inp")
                nc.sync.dma_start(out=inp_r[:], in_=inp_d.ap())
                w2_r = wp.tile([128, W2X], BF, name="w2r")
                for lo, hi in ((W2C0, W2C1), (W2C1, W2C2),
                               (W2C2, W2X)):
                    nc.sync.dma_start(out=w2_r[:, lo:hi],
                                      in_=w2_d.ap()[:, lo:hi])

                zt = [inp_r[:, ZT0:ZT0 + BL], inp_r[:, ZT1:ZT1 + BL]]
                zt2 = inp_r[:, ZT2:ZT2 + BL]      # replicated x4 leftover
                g1w_r = inp_r[0:GATE_H, G1W:G1W + GATE_H]
                g2w_r = inp_r[0:GATE_H, G2W:G2W + E]
                b2_r = w2_r[0:E, B2C:B2C + OUT_SIZE]
                # f32 biases travel inside the bf16 pack as raw bit pairs
                sm = inp_r[0:GATE_H, SMC:SMC + 24].bitcast(F32)
                g0b = sm[:, 0:1]
                b1_adj = sm[:, 1:2]          # g1_b - colsum(g1_w), host side
                adj2_col = sm[0:E, 2:3]      # g2_b - colsum(g2_w), host side
                adj2_row = sm[0:1, 3:3 + E]

                # ELU+1 = relu(x) + min(exp(x),1); the two pieces feed the
                # NEXT layer as two PSUM-accumulating matmuls (no combine on
                # the critical path); "+1" absorbed by host-adjusted biases.
                def elu_pieces(ps_in, bias, pref):
                    t_exp = wk.tile([GATE_H, BL], F32, name=f"{pref}_exp")
                    nc.scalar.activation(t_exp[:], ps_in, AF.Exp, bias=bias)
                    t_min = wk.tile([GATE_H, BL], BF, name=f"{pref}_min")
                    nc.vector.tensor_scalar(t_min[:], t_exp[:], 1.0, None,
                                            OP.min)
                    t_relu = wk.tile([GATE_H, BL], BF, name=f"{pref}_relu")
                    nc.vector.tensor_scalar(t_relu[:], ps_in, bias, 0.0,
                                            OP.add, OP.max)
                    return t_relu, t_min

                with tc.high_priority():
                    h0_ps = pg.tile([GATE_H, BL], F32, name="h0ps", tag="pg")
                    for i in range(2):
                        nc.tensor.matmul(
                            h0_ps[:],
                            inp_r[:, G0W + i * GATE_H:G0W + (i + 1) * GATE_H],
                            zt[i], start=(i == 0), stop=False)
                    nc.tensor.matmul(
                        h0_ps[:],
                        inp_r[0:32, G0W + 2 * GATE_H:G0W + 3 * GATE_H],
                        zt2[0:32, :], start=False, stop=True)
                    h0_a, h0_b = elu_pieces(h0_ps[:], g0b, "e0")

                    h1_ps = pg.tile([GATE_H, BL], F32, name="h1ps", tag="pg")
                    nc.tensor.matmul(h1_ps[:], g1w_r, h0_a[:],
                                     start=True, stop=False)
                    nc.tensor.matmul(h1_ps[:], g1w_r, h0_b[:],
                                     start=False, stop=True)
                    h1_a, h1_b = elu_pieces(h1_ps[:], b1_adj, "e1")

                    # exp(logits) in [b, 8] layout per chunk for per-partition
                    # scales (unnormalized; 1/sum applied at final eviction)
                    exp_sb = []    # (expc [128,8], rcp [128,1]) per chunk
                    for c in range(NCH):
                        lg_ps = pg.tile([128, E], F32, name="lgps", tag="pg")
                        nc.tensor.matmul(lg_ps[:],
                                         h1_a[:, c * 128:(c + 1) * 128],
                                         g2w_r, start=True, stop=False)
                        nc.tensor.matmul(lg_ps[:],
                                         h1_b[:, c * 128:(c + 1) * 128],
                                         g2w_r, start=False, stop=False)
                        nc.tensor.matmul(lg_ps[:], ones_row[:], adj2_row,
                                         start=False, stop=True)
                        expc = wk.tile([128, E], F32, name="expc")
                        sume = wk.tile([128, 1], F32, name="sume")
                        nc.scalar.activation(expc[:], lg_ps[:], AF.Exp,
                                             accum_out=sume[:])
                        rcp = wk.tile([128, 1], F32, name="rcp")
                        nc.vector.reciprocal(rcp[:], sume[:])
                        exp_sb.append((expc, rcp))

                    # ... and unnormalized exp(logits) in transposed [8, b]
                    # layout (for the mixed-bias matmul)
                    lgT_ps = pg.tile([E, BL], F32, name="lgTps", tag="pg")
                    nc.tensor.matmul(lgT_ps[:], g2w_r, h1_a[:],
                                     start=True, stop=False)
                    nc.tensor.matmul(lgT_ps[:], g2w_r, h1_b[:],
                                     start=False, stop=True)
                    expT_u = wk.tile([E, BL], BF, name="expTu")
                    nc.scalar.activation(expT_u[:], lgT_ps[:], AF.Exp,
                                         bias=adj2_col)

                # ---------------- expert layer + combine ----------------
                # Per batch-chunk: 4 pair matmuls per full K-chunk (stationary
                # zT slice shared), then ONE concurrent row-tiled wave for the
                # K=32 leftover; coef-scaled bf16 eviction; PE re-sum of 8
                # identity-matmuls + the mixed-bias matmul.
                out_sb = wk.tile([128, NCH * OUT_SIZE], BF, name="outsb")
                for c in range(NCH):
                    yps = []
                    for p in range(4):
                        yp = py.tile([128, 2 * OUT_SIZE], F32, name=f"yp{p}",
                                     tag="py")
                        yps.append(yp)
                    for i in range(2):
                        lhs = zt[i][:, c * 128:(c + 1) * 128]
                        for p in range(4):
                            nc.tensor.matmul(
                                yps[p][:], lhs,
                                w2_r[:, i * 2048 + p * 512:
                                     i * 2048 + (p + 1) * 512],
                                start=(i == 0), stop=False)
                    for p in range(4):
                        nc.tensor.matmul(
                            yps[p][:],
                            zt2[32 * p:32 * (p + 1),
                                c * 128:(c + 1) * 128],
                            w2_r[32 * p:32 * (p + 1), W2C2:W2C2 + 512],
                            start=False, stop=True,
                            tile_position=(32 * p, 0))

                    # pair-fused eviction: t_e0 = coef_e0*Y_e0 (ACT/DVE),
                    # then t_p = coef_e1*Y_e1 + t_e0 in ONE DVE
                    # scalar_tensor_tensor -- halves the PE re-sum fan-in
                    ys = []
                    for p in range(4):
                        e0, e1 = 2 * p, 2 * p + 1
                        s0 = exp_sb[c][0][:, e0:e0 + 1]
                        s1 = exp_sb[c][0][:, e1:e1 + 1]
                        t0 = wk.tile([128, OUT_SIZE], BF, name=f"yt{p}")
                        nc.scalar.activation(t0[:], yps[p][:, 0:OUT_SIZE],
                                             AF.Copy, scale=s0)
                        t1 = wk.tile([128, OUT_SIZE], BF, name=f"yu{p}")
                        nc.vector.scalar_tensor_tensor(
                            t1[:], yps[p][:, OUT_SIZE:2 * OUT_SIZE], s1,
                            t0[:], OP.mult, OP.add)
                        ys.append(t1)

                    out_ps = py.tile([128, OUT_SIZE], F32, name="outps",
                                     tag="py")
                    nc.tensor.matmul(out_ps[:],
                                     expT_u[:, c * 128:(c + 1) * 128],
                                     b2_r, start=True, stop=False)
                    for p in range(4):
                        nc.tensor.matmul(out_ps[:], ident[:], ys[p][:],
                                         start=False, stop=(p == 3))
                    # final eviction applies the softmax normalization 1/sum
                    nc.scalar.activation(
                        out_sb[:, c * OUT_SIZE:(c + 1) * OUT_SIZE],
                        out_ps[:], AF.Copy, scale=exp_sb[c][1][:])
                nc.scalar.dma_start(out=out_d.ap(), in_=out_sb[:])

    nc.finalize()
    return nc


def _get_nc(reps=1):
    key = ("nc", reps)
    if key not in _CACHE:
        _CACHE[key] = _build_nc(reps)
    return _CACHE[key]


def make_in_maps(z, g0_w, g0_b, g1_w, g1_b, g2_w, g2_b, w2, b2, **_unused):
    BF = ml_dtypes.bfloat16
    z = np.asarray(z, dtype=np.float32)
    g0_w = np.asarray(g0_w, dtype=np.float32)
    g1_w = np.asarray(g1_w, dtype=np.float32)
    g2_w = np.asarray(g2_w, dtype=np.float32)
    g0_b = np.asarray(g0_b, dtype=np.float32)
    g1_b = np.asarray(g1_b, dtype=np.float32)
    g2_b = np.asarray(g2_b, dtype=np.float32)
    w2 = np.asarray(w2, dtype=np.float32)
    b2 = np.asarray(b2, dtype=np.float32)

    smallp = np.zeros((GATE_H, 12), dtype=np.float32)
    smallp[:, 0] = g0_b
    # adjusted biases absorb the ELU "+1" offset of the previous layer
    smallp[:, 1] = g1_b - g1_w.sum(axis=0)
    adj2 = g2_b - g2_w.sum(axis=0)
    smallp[0:E, 2] = adj2
    smallp[0, 3:3 + E] = adj2
    smallp_bits = np.ascontiguousarray(smallp).view(np.uint16).view(BF)

    # w2 pack: w2t [288, 2048] column-major by expert
    w2t = np.ascontiguousarray(w2.transpose(1, 0, 2)).reshape(IN_SIZE, 2048)
    w2p = np.zeros((128, W2X), dtype=np.float32)
    for i in range(2):
        for p in range(4):
            w2p[:, i * 2048 + p * 512:i * 2048 + (p + 1) * 512] = \
                w2t[i * 128:(i + 1) * 128, p * 512:(p + 1) * 512]
    for p in range(4):
        w2p[32 * p:32 * (p + 1), W2C2:W2C2 + 512] = \
            w2t[256:288, p * 512:(p + 1) * 512]
    w2p[0:E, B2C:B2C + OUT_SIZE] = b2

    inp_shared = np.zeros((128, INPX), dtype=np.float32)
    inp_shared[:, G0W:G0W + GATE_H] = g0_w[0:128]
    inp_shared[:, G0W + GATE_H:G0W + 2 * GATE_H] = g0_w[128:256]
    inp_shared[0:32, G0W + 2 * GATE_H:G0W + 3 * GATE_H] = g0_w[256:288]
    inp_shared[0:GATE_H, G1W:G1W + GATE_H] = g1_w
    inp_shared[0:GATE_H, G2W:G2W + E] = g2_w

    shared = {"w2p": w2p.astype(BF)}
    maps = []
    for c in range(N_CORES):
        zT = z[c * BL:(c + 1) * BL].T                      # [288, 256]
        inp = inp_shared.copy()
        inp[:, ZT0:ZT0 + BL] = zT[0:128]
        inp[:, ZT1:ZT1 + BL] = zT[128:256]
        for p in range(4):
            inp[32 * p:32 * (p + 1), ZT2:ZT2 + BL] = zT[256:288]
        inp_bf = inp.astype(BF)
        # raw f32 bias bytes ride along as bf16 bit pairs (no rounding)
        inp_bf[0:GATE_H, SMC:SMC + 24] = smallp_bits
        maps.append(dict(shared, inp=inp_bf))
    return maps


def unpack_out(res_list):
    full = np.empty((B, OUT_SIZE), dtype=np.float32)
    for c in range(N_CORES):
        packed = np.asarray(res_list[c]["outp"], dtype=np.float32)
        for ch in range(NCH):
            full[c * BL + ch * 128:c * BL + (ch + 1) * 128] = \
                packed[:, ch * OUT_SIZE:(ch + 1) * OUT_SIZE]
    return full


def kernel(**inputs):
    from concourse.bass_utils import run_bass_kernel_spmd

    nc = _get_nc()
    in_maps = make_in_maps(**inputs)
    res = run_bass_kernel_spmd(nc, in_maps, list(range(N_CORES)))
    return unpack_out(res.results)
